# revision 32
# baseline (speedup 1.0000x reference)
"""Bass/Trainium2 kernel for the additive-attention nn.Module.

Computation (per batch b):
    energy[s, a] = tanh( enc[b,s,:] @ W_enc[a,:] + hidden[b,:] @ W_dec[a,:] + b_attn[a] )
    scores[s]    = energy[s, :] @ v
    w            = softmax(scores)
    ctx[b, :]    = w @ enc[b]

Sharding: data-parallel over batch across 8 NeuronCores (8 batches/core);
weights replicated.

v2 layout strategy: the host pre-transposes enc ([B, E, S]) and W ([F, A]),
so the device performs ZERO layout transposes for the big GEMM:
  - energy^T runs as psum[a-block, s-chunk] += W_encT[e, a-block].T @
    encT[e, s-chunk]; both operands are direct DMA slices of the host
    layouts (fp32r, 1 PE cycle/row).
  - hb[a, b] = hidden @ W_dec + b_attn is a tiny [8, A] PE GEMM, transposed
    on-chip (8 small PE transposes) into per-partition bias columns.
  - ScalarEngine computes tanh(energy + hb[:, b]) straight from PSUM in one
    pass (per-partition bias), PE contracts with v into scores rows.
  - softmax on the [1, S] scores row (ACT exp with accum_out sum).
  - ctx = Σ_s w_s enc[s, e] is a free-axis reduce over the encT tiles:
    Pool multiplies by the broadcast exp-weight row, DVE reduce_sum gives
    ctxT columns; scaled by 1/sum at the tail, PE-transposed to [b, e] rows.
PE work is ~89% the irreducible energy GEMM.
"""

import sys

if "/opt/trn_rl_repo" not in sys.path:
    sys.path.insert(0, "/opt/trn_rl_repo")

import numpy as np

B, S, DEC, ENC, ATTN = 64, 1024, 1024, 1024, 1024
N_CORES = 8
B_LOC = B // N_CORES

_CACHE = {}


def build_nc(B_loc=B_LOC, S_=S, E=ENC, A=ATTN, D=DEC, loop_n=None, ablate=None):
    from contextlib import ExitStack

    import concourse.bacc as bacc
    import concourse.tile as tile
    from concourse import mybir
    from concourse.bass import ts
    from concourse.masks import make_identity

    P = 128
    F32 = mybir.dt.float32
    F32R = mybir.dt.float32r
    F16 = mybir.dt.float16
    AF = mybir.ActivationFunctionType
    AX = mybir.AxisListType
    ALU = mybir.AluOpType

    n_ab = A // P             # a-blocks (energy psum partition dim)
    n_eb = E // P             # e-blocks (contraction)
    n_db = D // P
    SCW = 512                 # s-chunk width (fp32 moving max)
    n_sc = S_ // SCW

    nc = bacc.Bacc("TRN2", target_bir_lowering=False, debug=False)
    encT_d = nc.dram_tensor("encT", [B_loc, E, S_], F16, kind="ExternalInput")
    we_d = nc.dram_tensor("wenc", [E, A], F16, kind="ExternalInput")
    wd_d = nc.dram_tensor("wdec", [D, A], F16, kind="ExternalInput")
    hidT_d = nc.dram_tensor("hidT", [D, B_loc], F16, kind="ExternalInput")
    b_d = nc.dram_tensor("b_attn", [A], F32, kind="ExternalInput")
    v_d = nc.dram_tensor("v", [A], F32, kind="ExternalInput")
    ctx_d = nc.dram_tensor("ctx", [B_loc, E], F32, kind="ExternalOutput")

    with tile.TileContext(nc) as tc:
        with ExitStack() as ctx:
            const = ctx.enter_context(tc.tile_pool(name="const", bufs=1))
            wenc_p = ctx.enter_context(tc.tile_pool(name="wenc", bufs=2))
            wdec_p = ctx.enter_context(tc.tile_pool(name="wdec", bufs=2))
            encT_p = ctx.enter_context(tc.tile_pool(name="encT", bufs=3))
            th_p = ctx.enter_context(tc.tile_pool(name="th", bufs=6))
            cx_p = ctx.enter_context(tc.tile_pool(name="cx", bufs=2))
            soft_p = ctx.enter_context(tc.tile_pool(name="soft", bufs=2))
            ewbc_p = ctx.enter_context(tc.tile_pool(name="ewbc", bufs=2))
            psE = ctx.enter_context(tc.tile_pool(name="psE", bufs=6, space="PSUM"))
            psS = ctx.enter_context(tc.tile_pool(name="psS", bufs=2, space="PSUM"))

            if loop_n is not None:
                ctx.enter_context(tc.For_i(0, loop_n, 1))

            # ---- small DMAs + constants ----
            hidT = const.tile([P, n_db, B_loc], F16, name="hidT")
            nc.sync.dma_start(
                hidT[:], hidT_d.ap().rearrange("(db p) b -> p db b", p=P)
            )
            b_row = const.tile([1, A], F32, name="b_row")
            nc.sync.dma_start(b_row[:], b_d.ap().rearrange("(o a) -> o a", o=1))
            vcol = const.tile([P, n_ab], F32R, name="vcol")
            nc.sync.dma_start(
                vcol[:], v_d.ap().rearrange("(j p) -> p j", p=P).bitcast(F32R)
            )
            ones8 = const.tile([1, B_loc], F32, name="ones8")
            nc.gpsimd.memset(ones8[:], 1.0)
            ident0 = const.tile([P, P], F32, name="ident0")
            make_identity(nc, ident0[:])
            ident = const.tile([P, P], F32R, name="ident")
            nc.vector.tensor_copy(ident[:], ident0[:])

            # ---- hb = hidden @ W_dec + b_attn -> [B_loc, A] psum, then
            # transpose into per-partition bias columns hb_all[:, ab, b].
            # Emitted before the bulk W_enc/encT DMAs so W_dec lands first
            # and the PE-stream-head hb matmuls don't delay the energy GEMM.
            hb8 = const.tile([B_loc, A], F32R, name="hb8")
            wd = wdec_p.tile([P, n_db, A], F16, tag="wd", name="wd")
            nc.sync.dma_start(
                wd[:], wd_d.ap().rearrange("(db p) a -> p db a", p=P)
            )
            for ac in range(A // SCW):
                ph = psS.tile([B_loc, SCW], F32, tag="s", name=f"ph{ac}")
                for db in range(n_db):
                    nc.tensor.matmul(
                        ph[:],
                        hidT[:, db],
                        wd[:, db, ts(ac, SCW)],
                        start=(db == 0),
                        stop=False,
                        skip_group_check=True,
                    )
                nc.tensor.matmul(
                    ph[:],
                    ones8[:],
                    b_row[0:1, ts(ac, SCW)],
                    start=False,
                    stop=True,
                    skip_group_check=True,
                )
                nc.vector.tensor_copy(hb8[:, ts(ac, SCW)], ph[:])
            hb_all = const.tile([P, n_ab, B_loc], F32, name="hb_all")
            for ab in range(n_ab):
                pt = psS.tile([P, B_loc], F32R, tag="s", name=f"phb{ab}")
                nc.tensor.transpose(
                    pt[:], hb8[:, ts(ab, P)], ident[0:B_loc, 0:B_loc]
                )
                nc.vector.tensor_copy(hb_all[:, ab], pt[:].bitcast(F32))

            # ---- bulk DMAs: W_enc (one DMA) + per-batch encT (one DMA) ----
            et_tiles = {}

            def load_enc(b):
                t = encT_p.tile([P, n_eb, S_], F16, tag="encT", name=f"enc_{b}")
                nc.sync.dma_start(
                    t[:], encT_d.ap()[b].rearrange("(eb p) s -> p eb s", p=P)
                )
                et_tiles[b] = t

            we = wenc_p.tile([P, n_eb, A], F16, tag="we", name="we")
            nc.sync.dma_start(we[:], we_d.ap().rearrange("(eb p) a -> p eb a", p=P))
            load_enc(0)

            # persistent ctxT columns [e-block, b] + per-batch 1/sum row
            ctxT = [
                const.tile([P, B_loc], F32, name=f"ctxT{eb}") for eb in range(n_eb)
            ]
            rc_row = const.tile([1, B_loc], F32, name="rc_row")

            # ---- per-batch softmax + context (emitted one batch behind) ----
            scores_rows = {}

            def emit_softmax_ctx(b):
                row = scores_rows[b][0:1, :]
                nm = soft_p.tile([1, 1], F32, tag="nm", name=f"nm{b}")
                nc.vector.reduce_max(nm[:], row, axis=AX.X, negate=True)
                ew_row = soft_p.tile([1, S_], F16, tag="ew", name=f"ew{b}")
                esum = soft_p.tile([1, 1], F32, tag="esum", name=f"esum{b}")
                nc.scalar.activation(
                    ew_row[:], row, AF.Exp, bias=nm[0:1, 0:1], accum_out=esum[:],
                )
                nc.vector.reciprocal(rc_row[0:1, b:b + 1], esum[:])
                ew_bc = ewbc_p.tile([P, S_], F16, tag="ewbc", name=f"ewbc{b}")
                nc.gpsimd.partition_broadcast(ew_bc[:], ew_row[:])
                # ctx: fp16 mult + reduce on DVE (2x packed modes) per e-block
                for eb in range(n_eb):
                    cx = cx_p.tile([P, S_], F16, tag="cx", name=f"cx{b}_{eb}")
                    nc.vector.tensor_tensor(
                        cx[:], et_tiles[b][:, eb], ew_bc[:], op=ALU.mult,
                    )
                    nc.vector.reduce_sum(ctxT[eb][:, b:b + 1], cx[:], axis=AX.X)
                del et_tiles[b]

            # ---- main loop: energy GEMM + tanh + scores, pipelined ----
            # score matmuls trail their tanh by SCORE_LAG energy groups (a
            # FIFO of closures, popped one per group, crossing batch bounds)
            from collections import deque
            pending_soft = None
            score_q = deque()
            SCORE_LAG = 1

            for b in range(B_loc):
                scores_row = soft_p.tile([1, S_], F32, tag="scores", name=f"scores{b}")
                scores_rows[b] = scores_row
                ps_ss = [None] * n_sc

                def mk_score(b, ab, ths, ps_ss, scores_row):
                    def emit():
                        for sc in range(n_sc):
                            if ps_ss[sc] is None:
                                ps_ss[sc] = psS.tile(
                                    [1, SCW], F32, tag="s", name=f"pss{b}_{sc}"
                                )
                            nc.tensor.matmul(
                                ps_ss[sc][:],
                                vcol[:, ab:ab + 1],
                                ths[sc][:],
                                start=(ab == 0),
                                stop=(ab == n_ab - 1),
                                skip_group_check=True,
                            )
                        if ab == n_ab - 1:
                            for sc in range(n_sc):
                                nc.vector.tensor_copy(
                                    scores_row[0:1, ts(sc, SCW)], ps_ss[sc][:]
                                )
                    return emit

                for ab in range(n_ab):
                    # both s-chunks consume each stationary W slice
                    # back-to-back so redundant weight loads can be elided
                    pss = [
                        psE.tile([P, SCW], F32, tag="e", name=f"pse{b}_{sc}_{ab}")
                        for sc in range(n_sc)
                    ]
                    for eb in range(n_eb):
                        for sc in range(n_sc):
                            nc.tensor.matmul(
                                pss[sc][:],
                                we[:, eb, ts(ab, P)],
                                et_tiles[b][:, eb, ts(sc, SCW)],
                                start=(eb == 0),
                                stop=(eb == n_eb - 1),
                                skip_group_check=True,
                            )
                    if ablate != "gemm":
                        ths = []
                        for sc in range(n_sc):
                            th = th_p.tile(
                                [P, SCW], F32R, tag="th", name=f"th{b}_{sc}_{ab}"
                            )
                            nc.scalar.activation(
                                th[:], pss[sc][:], AF.Tanh,
                                bias=hb_all[:, ab, b:b + 1],
                            )
                            ths.append(th)
                        score_q.append(mk_score(b, ab, ths, ps_ss, scores_row))
                        while len(score_q) > SCORE_LAG:
                            score_q.popleft()()
                    if ab == 1:
                        if pending_soft is not None and ablate is None:
                            emit_softmax_ctx(pending_soft)
                        pending_soft = None
                        if b + 1 < B_loc:
                            load_enc(b + 1)
                if ablate is None:
                    pending_soft = b
                else:
                    if b in et_tiles:
                        del et_tiles[b]
            while score_q:
                score_q.popleft()()
            if pending_soft is not None and ablate is None:
                emit_softmax_ctx(pending_soft)

            # ---- tail: scale ctxT by 1/sum, transpose to [b, e] rows, DMA ----
            crows = const.tile([B_loc, E], F32, name="crows")
            if ablate is not None:
                nc.gpsimd.memset(crows[:], 0.0)
            else:
                rc_bc = const.tile([P, B_loc], F32, name="rc_bc")
                nc.gpsimd.partition_broadcast(rc_bc[:], rc_row[:])
                for eb in range(n_eb):
                    cts = soft_p.tile([P, B_loc], F32R, tag="cts", name=f"cts{eb}")
                    nc.vector.tensor_tensor(
                        cts[:], ctxT[eb][:], rc_bc[:], op=ALU.mult
                    )
                    ctr = psS.tile([B_loc, P], F32R, tag="s", name=f"ctr{eb}")
                    nc.tensor.transpose(ctr[:], cts[:], ident[:])
                    nc.vector.tensor_copy(crows[:, ts(eb, P)], ctr[:].bitcast(F32))
            nc.sync.dma_start(ctx_d.ap(), crows[:])

    nc.compile()
    return nc


def _get_nc():
    key = (B_LOC, S, ENC, ATTN, DEC)
    if key not in _CACHE:
        _CACHE[key] = build_nc(*key)
    return _CACHE[key]


def _prep(hidden, encoder_outputs, W_attn, b_attn, v):
    hidden = np.asarray(hidden, dtype=np.float32)
    enc = np.asarray(encoder_outputs, dtype=np.float32)
    W = np.asarray(W_attn, dtype=np.float32)
    b = np.ascontiguousarray(np.asarray(b_attn, dtype=np.float32))
    vv = np.ascontiguousarray(np.asarray(v, dtype=np.float32))

    encT = np.ascontiguousarray(enc.transpose(0, 2, 1).astype(np.float16))
    WT = np.ascontiguousarray(W.T)                               # [F, A]
    wdec = np.ascontiguousarray(WT[:DEC].astype(np.float16))    # [D, A]
    wenc = np.ascontiguousarray(WT[DEC:].astype(np.float16))     # [E, A]
    hidT = np.ascontiguousarray(hidden.T.astype(np.float16))    # [D, B]
    return encT, wenc, wdec, hidT, b, vv


def kernel(hidden, encoder_outputs, W_attn, b_attn, v):
    from concourse.bass_utils import run_bass_kernel_spmd

    encT, wenc, wdec, hidT, b, vv = _prep(
        hidden, encoder_outputs, W_attn, b_attn, v
    )

    nc = _get_nc()
    in_maps = [
        {
            "encT": encT[c * B_LOC:(c + 1) * B_LOC],
            "wenc": wenc,
            "wdec": wdec,
            "hidT": np.ascontiguousarray(hidT[:, c * B_LOC:(c + 1) * B_LOC]),
            "b_attn": b,
            "v": vv,
        }
        for c in range(N_CORES)
    ]
    res = run_bass_kernel_spmd(nc, in_maps, core_ids=list(range(N_CORES)))
    out = np.concatenate([res.results[c]["ctx"] for c in range(N_CORES)], axis=0)
    return out.reshape(B, 1, ENC).astype(np.float32)
